# revision 16
# baseline (speedup 1.0000x reference)
"""Trainium2 Bass kernel: single attention head (B=8, S=2048, E=1024, H=64).

Sharding: data-parallel over batch -- each of the 8 NeuronCores computes one
batch element's full attention. No collectives; every HBM byte read once.

v12 design (merged constants, max score pairing, qh0-first AV order):
  - v11 post-mortem (82.5us): exp#0 fired at 28us (predicted 18.5) because
    the five tiny weight/bias DMAs each split into 16 per-SDMA-engine
    descriptors whose completion increments trickled out round-robin
    behind the megabyte input packets -- the DVE head-of-line waited on
    bq's sem until 25.9us. Mid-kernel exp gaps (~0.4-2.3us every 2 slabs)
    came from PE in-order queuing: with PE work ~= ACT work (both ~36us),
    AV/proj matmuls ahead of the next score pair push exp out.
  - Constants: ONE f16 blob DMA (wq | wk | wv | bq | bv, [128, 1538]) on
    the scalar ring, done by ~11us with one sem. The duplicated-Wq
    stationary is built on-chip with two cheap DVE copies.
  - Score pairing everywhere timing allows: 14 row-tiled pairs + 4
    singles (PE score cost 10.4 -> 7.7us). Slab order interleaves qh1
    pairs in front of the k3-gated cells so exp never waits on the k3
    arrival; all 16 qh0 cells still finish early so finalize(0/1) and
    the oa-bank recycle overlap the remaining qh1 exps.
  - PSUM (q-half-sequential AV accumulator, as v11): 2x 2-bank f32 score
    slots + 2x 1-bank projection slots + 2-bank [65,1024] AV accumulator
    recycled across q halves = 8 banks. Projections never steal score
    slots.
  - v^T -> vaug via PE transpose + DVE copy (off the DMA rings). Output
    DMAs on the sync ring; scalar engine runs pure ACTIVATE after ~11us.
  - As earlier: transposed scores (keys on partitions), rowsums ride the
    ones column of the AV stationary, bk cancels in softmax, bq/bv fold
    into evacuations, exp scale=1/8 fused into ACTIVATE.
"""

import numpy as np

import concourse.bass as bass  # noqa: F401  (engine namespaces live on nc)
import concourse.mybir as mybir
import concourse.tile as tile
from concourse import bacc
from concourse.bass_utils import run_bass_kernel_spmd
from concourse.masks import make_identity

B, S, E, H = 8, 2048, 1024, 64
EC = E // 128    # contraction chunks (128 partitions each)
KB = 512         # kv block columns
NKB = S // KB    # 4 kv blocks
NT = S // 128    # key tiles
CW = EC * 64     # one weight matrix's columns in the const blob (512)
F16 = mybir.dt.float16
F32 = mybir.dt.float32

_CACHE = {}


def _build_nc():
    nc = bacc.Bacc(None)
    xq = nc.declare_dram_parameter("xq", [128, 2, 2, EC, KB], F16, isOutput=False)
    xk = nc.declare_dram_parameter("xk", [128, NKB, EC, KB], F16, isOutput=False)
    xv = nc.declare_dram_parameter("xv", [128, NKB, EC, KB], F16, isOutput=False)
    cst = nc.declare_dram_parameter("cst", [128, 3 * CW + 2], F16, isOutput=False)
    out = nc.declare_dram_parameter("out", [S, H], F32, isOutput=True)

    Exp = mybir.ActivationFunctionType.Exp

    with tile.TileContext(nc) as tc:
        with tc.tile_pool(name="const", bufs=1) as const, \
             tc.tile_pool(name="xkp", bufs=4) as xkp, \
             tc.tile_pool(name="xvp", bufs=4) as xvp, \
             tc.tile_pool(name="ptp", bufs=16) as ptp, \
             tc.tile_pool(name="vtp", bufs=2) as vtp, \
             tc.tile_pool(name="p5sb", bufs=2) as p5sb, \
             tc.tile_pool(name="psp", bufs=2, space="PSUM") as psp, \
             tc.tile_pool(name="pjp", bufs=2, space="PSUM") as pjp, \
             tc.tile_pool(name="oap", bufs=1, space="PSUM") as oap:

            # ---- constants: ONE blob DMA on the scalar ring ----
            cst_t = const.tile([128, 3 * CW + 2], F16, name="cst_t")
            nc.scalar.dma_start(out=cst_t[:], in_=cst[:])
            cst_r = cst_t[:, 0:3 * CW].rearrange(
                "p (m c w) -> p m c w", m=3, c=EC, w=64)
            bias32 = const.tile([128, 2], F32, name="bias32")
            nc.vector.tensor_copy(bias32[:], cst_t[:, 3 * CW:3 * CW + 2])
            bq_t = bias32[:, 0:1]
            bv_t = bias32[:, 1:2]

            # duplicated-Wq stationary built on-chip (2 cheap DVE copies)
            wqd_t = const.tile([128, EC, 128], F16, name="wqd_t")
            nc.vector.tensor_copy(wqd_t[:, :, 0:64], cst_r[:, 0])
            nc.vector.tensor_copy(wqd_t[:, :, 64:128], cst_r[:, 0])

            def wk_c(c):
                return cst_r[:, 1, c]

            def wv_c(c):
                return cst_r[:, 2, c]

            qt = const.tile([128, S], F16, name="qt")     # q^T in BOTH halves
            kt = const.tile([128, S], F16, name="kt")     # k^T: half (jb%2)
            xqt = const.tile([128, 2, 2, EC, KB], F16, name="xqt")
            vaug = const.tile([128, NT, 80], F16, name="vaug")
            oasb = const.tile([65, S], F16, name="oasb")
            ident = const.tile([128, 128], F16, name="ident")
            osb_all = const.tile([128, NT, H], F32, name="osb_all")

            make_identity(nc, ident[:])
            nc.vector.memset(vaug[:, :, 64], 1.0)

            # AV accumulator: ONE q half at a time ([65, 1024] = 2 banks);
            # the qh1 tile reuses the banks after finalize of qh0 (bufs=1)
            oa_t = [None, None]
            oa_t[0] = oap.tile([65, S // 2], F32, tag="oa", name="oa0")

            # ---- input DMAs (sync HWDGE FIFO -- executes in this order) ----
            xkts, xvts = [], []

            def fetch(which, jb):
                if which == "k":
                    xt = xkp.tile([128, EC, KB], F16, tag="xk", name=f"xkt{jb}")
                    nc.sync.dma_start(out=xt[:], in_=xk[:, jb])
                    xkts.append(xt)
                else:
                    xt = xvp.tile([128, EC, KB], F16, tag="xv", name=f"xvt{jb}")
                    nc.sync.dma_start(out=xt[:], in_=xv[:, jb])
                    xvts.append(xt)

            nc.sync.dma_start(out=xqt[:, 0, 0], in_=xq[:, 0, 0])
            fetch("k", 0)
            nc.sync.dma_start(out=xqt[:, 0, 1], in_=xq[:, 0, 1])
            fetch("k", 1)
            fetch("k", 2)
            nc.sync.dma_start(out=xqt[:, 1, 0], in_=xq[:, 1, 0])
            nc.sync.dma_start(out=xqt[:, 1, 1], in_=xq[:, 1, 1])
            fetch("v", 0)
            fetch("k", 3)
            fetch("v", 1)
            fetch("v", 2)
            fetch("v", 3)

            # ---- PE warm-keeper: spans the preamble + xq0s0 DMA head so
            # the HAM clock gate is at 2.4 GHz for the first projection.
            wslot = psp.tile([128, 128], F32, tag="ps", name="warm")
            for _ in range(52):
                nc.tensor.matmul(
                    wslot[:], ident[:], ident[:],
                    start=True, stop=True, skip_group_check=True)

            # ---- AV queue: always-ready filler matmuls ----
            pts = [None] * NT           # per-tile exp(S^T) SBUF tiles
            av_ready = []
            av_bank_count = [0] * 4
            vdone = set()               # blocks whose vaug tiles exist

            def emit_av(n):
                # drain up to n AV cells; eligible = vaug tile exists AND
                # the cell's q-half accumulator currently exists
                emitted = []
                for cell in av_ready:
                    if n <= 0:
                        break
                    t, qh = cell
                    if (t // 4) not in vdone or oa_t[qh] is None:
                        continue
                    for sg in range(2):
                        seg = 2 * qh + sg
                        cnt = av_bank_count[seg]
                        nc.tensor.matmul(
                            oa_t[qh][:, sg * 512:(sg + 1) * 512],
                            vaug[:, t, 0:65],
                            pts[t][:, qh * 1024 + sg * 512:
                                   qh * 1024 + (sg + 1) * 512],
                            start=(cnt == 0), stop=(cnt == NT - 1),
                            skip_group_check=True)
                        av_bank_count[seg] = cnt + 1
                    emitted.append(cell)
                    n -= 1
                for cell in emitted:
                    av_ready.remove(cell)

            def qproj_seg(qh, sg):
                # one 512-col q segment: 8 chunk matmuls into a proj slot,
                # then evacuate (+bq) to qt
                ps = pjp.tile([128, KB], F32, tag="pj", name=f"pq{qh}{sg}")
                for c in range(EC):
                    nc.tensor.matmul(
                        ps[:], wqd_t[:, c, :], xqt[:, qh, sg, c, :],
                        start=(c == 0), stop=(c == EC - 1),
                        skip_group_check=True)
                nc.vector.tensor_scalar_add(
                    qt[:, qh * 1024 + sg * 512:qh * 1024 + (sg + 1) * 512],
                    ps[:], bq_t)

            def proj_pass(specs):
                """One col-tiled k/v projection pass in a proj slot.
                specs: list of ("k"|"v", jb, row0), len 1 or 2; row0 in
                {0, 64}, distinct within a pass. k rows MUST equal
                (jb%2)*64 to match kt; v rows are free."""
                ps = pjp.tile([128, KB], F32, tag="pj",
                              name="pkv" + "_".join(f"{w}{j}" for w, j, _ in specs))
                for c in range(EC):
                    for which, jb, r0 in specs:
                        w = wk_c(c) if which == "k" else wv_c(c)
                        x = xkts[jb] if which == "k" else xvts[jb]
                        nc.tensor.matmul(
                            ps[r0:r0 + 64, :], w, x[:, c, :],
                            start=(c == 0), stop=(c == EC - 1),
                            skip_group_check=True)
                vjobs = []
                for which, jb, r0 in specs:
                    if which == "k":
                        assert r0 == (jb % 2) * 64
                        nc.vector.tensor_copy(
                            kt[r0:r0 + 64, jb * KB:(jb + 1) * KB],
                            ps[r0:r0 + 64, :])
                    else:
                        vtb = vtp.tile([128, KB], F16, tag="vt",
                                       name=f"vtb{jb}")
                        nc.vector.tensor_scalar_add(
                            vtb[r0:r0 + 64, :], ps[r0:r0 + 64, :],
                            bv_t[r0:r0 + 64])
                        vjobs.append((jb, r0, vtb))
                for jb, r0, vtb in vjobs:
                    # v^T [64, 512] -> vaug 4x[128, 64] via PE transpose
                    # (stays off the DMA rings); trs2 takes a proj slot
                    trs2 = pjp.tile([128, 4, H], F16, tag="pj",
                                    name=f"vtr{jb}")
                    for j in range(4):
                        nc.tensor.transpose(
                            trs2[:, j, 0:H],
                            vtb[r0:r0 + 64, j * 128:(j + 1) * 128],
                            ident[r0:r0 + 64, r0:r0 + 64])
                    for j in range(4):
                        nc.vector.tensor_copy(
                            vaug[:, 4 * jb + j, 0:H], trs2[:, j, 0:H])
                    vdone.add(jb)

            def score_seg(t, qh, sl, sg, exp=True):
                # one N=512 score matmul (+ optional FD-512 exp) for seg sg
                g = ((t // 4) % 2) * 64
                nc.tensor.matmul(
                    sl[:, sg * 512:(sg + 1) * 512],
                    kt[g:g + 64, t * 128:(t + 1) * 128],
                    qt[g:g + 64, qh * 1024 + sg * 512:
                       qh * 1024 + (sg + 1) * 512],
                    start=True, stop=True, skip_group_check=True)
                if exp:
                    nc.scalar.activation(
                        pts[t][:, qh * 1024 + sg * 512:
                               qh * 1024 + (sg + 1) * 512],
                        sl[:, sg * 512:(sg + 1) * 512], Exp, scale=0.125)

            def slab_tile(t, qh):
                if pts[t] is None:
                    pts[t] = ptp.tile([128, S], F16, tag="pt", name=f"pt{t}")
                return psp.tile([128, 1024], F32, tag="ps", name=f"s{t}_{qh}")

            def score_slabs(cells, av=0):
                """Scores + exp for 1 or 2 (tile, qh) cells: 2x N=512
                matmuls into a 2-bank f32 slab + one FD-1024 exp each.
                A len-2 list must have opposite kt-half parity; its four
                matmuls interleave seg-by-seg so the two cells run
                concurrently on the PE via row tiling."""
                work = [(t, qh, slab_tile(t, qh)) for t, qh in cells]
                for sg in range(2):
                    for t, qh, sl in work:
                        score_seg(t, qh, sl, sg, exp=False)
                for t, qh, sl in work:
                    nc.scalar.activation(
                        pts[t][:, qh * 1024:(qh + 1) * 1024], sl[:],
                        Exp, scale=0.125)
                    av_ready.append((t, qh))
                if av:
                    emit_av(av)

            # ---- finalize: transpose, normalize, store (out on sync ring)
            out_r = out[:].rearrange("(t p) h -> p t h", p=128)

            def finalize_chunk(cq):
                nc.vector.tensor_copy(
                    oasb[:, cq * 512:(cq + 1) * 512],
                    oa_t[cq // 2][:, (cq % 2) * 512:(cq % 2 + 1) * 512])
                trs = psp.tile([128, 4, 66], F16, tag="ps", name=f"trs{cq}")
                for jj in range(4):
                    j = cq * 4 + jj
                    nc.tensor.transpose(
                        trs[:, jj, 0:65], oasb[:, j * 128:(j + 1) * 128],
                        ident[0:65, 0:65])
                rc = p5sb.tile([128, 4], F32, tag="rc", name=f"rc{cq}")
                nc.vector.reciprocal(rc[:], trs[:, :, 64])
                for jj in range(4):
                    j = cq * 4 + jj
                    nc.vector.tensor_scalar(
                        osb_all[:, j, :], trs[:, jj, 0:64], rc[:, jj:jj + 1],
                        None, op0=mybir.AluOpType.mult)
                nc.sync.dma_start(
                    out=out_r[:, cq * 4:(cq + 1) * 4, :],
                    in_=osb_all[:, cq * 4:(cq + 1) * 4, :])

            # ---- schedule (program order ~= per-engine issue order) ----
            # exp slab cadence ~1.15us; stream arrival gates annotated.
            qproj_seg(0, 0)                    # xq0s0
            proj_pass([("k", 0, 0)])           # k0
            sl00 = slab_tile(0, 0)             # cell (0,0) split by seg:
            score_seg(0, 0, sl00, 0)           # exp starts before xq0s1
            qproj_seg(0, 1)                    # xq0s1
            score_seg(0, 0, sl00, 1)
            av_ready.append((0, 0))
            score_slabs([(1, 0)])
            proj_pass([("k", 1, 64)])          # k1
            score_slabs([(4, 0), (2, 0)])      # row-tiled pairs (h1 x h0)
            proj_pass([("k", 2, 0)])           # k2
            score_slabs([(5, 0), (3, 0)])
            score_slabs([(6, 0), (8, 0)])
            score_slabs([(7, 0), (9, 0)])
            qproj_seg(1, 0)                    # xq1s0
            qproj_seg(1, 1)                    # xq1s1
            score_slabs([(4, 1), (0, 1)])
            proj_pass([("v", 0, 0)])           # v0
            score_slabs([(5, 1), (1, 1)], av=2)
            proj_pass([("k", 3, 64)])          # k3
            score_slabs([(6, 1), (8, 1)], av=2)
            score_slabs([(7, 1), (9, 1)], av=2)
            proj_pass([("v", 1, 0)])           # v1
            score_slabs([(12, 0), (10, 0)], av=2)
            score_slabs([(13, 0), (11, 0)], av=2)
            proj_pass([("v", 2, 0)])           # v2
            score_slabs([(14, 0), (2, 1)], av=3)
            proj_pass([("v", 3, 64)])          # v3
            score_slabs([(15, 0), (3, 1)], av=3)
            # all 16 qh0 cells exp'd; drain their AVs (emit_av auto-skips
            # qh1 until its accumulator exists), finalize chunks 0/1 under
            # the remaining exps, then recycle the banks for q half 1
            emit_av(6)
            finalize_chunk(0)
            finalize_chunk(1)
            oa_t[1] = oap.tile([65, S // 2], F32, tag="oa", name="oa1")
            score_slabs([(12, 1), (10, 1)], av=3)
            score_slabs([(13, 1), (11, 1)], av=3)
            score_slabs([(14, 1)], av=3)
            score_slabs([(15, 1)], av=3)
            emit_av(16)
            finalize_chunk(2)
            finalize_chunk(3)

    nc.finalize()
    return nc


def get_nc():
    if "nc" not in _CACHE:
        _CACHE["nc"] = _build_nc()
    return _CACHE["nc"]


def _stage_x(x, nblk, cb):
    # [S, E] f32 -> [128, nblk, EC, cb] f16 with [p, b, c, s] = x[b*cb+s, c*128+p]
    xt = np.ascontiguousarray(x.T.astype(np.float16))          # [E, S]
    xt = xt.reshape(EC, 128, nblk, cb).transpose(1, 2, 0, 3)   # [p, b, c, s]
    return np.ascontiguousarray(xt)


def _stage_xq(x):
    # [S, E] f32 -> [128, 2, 2, EC, 512] f16, seg-major:
    # [p, h, s, c, s2] = x[h*1024 + s*512 + s2, c*128 + p]
    xt = np.ascontiguousarray(x.T.astype(np.float16))          # [E, S]
    xt = xt.reshape(EC, 128, 2, 2, KB).transpose(1, 2, 3, 0, 4)
    return np.ascontiguousarray(xt)


def _stage_w(w):
    # [E, 64] -> [128, EC*64] f16 with [p, c*64+j] = w[c*128+p, j]
    wh = np.asarray(w, np.float32).astype(np.float16)
    return wh.reshape(EC, 128, 64).transpose(1, 0, 2).reshape(128, CW)


def make_in_maps(inputs):
    q = np.asarray(inputs["query"], np.float32)
    k = np.asarray(inputs["key_"], np.float32)
    v = np.asarray(inputs["value"], np.float32)
    cst = np.zeros((128, 3 * CW + 2), np.float16)
    cst[:, 0:CW] = _stage_w(inputs["Wq"])
    cst[:, CW:2 * CW] = _stage_w(inputs["Wk"])
    cst[:, 2 * CW:3 * CW] = _stage_w(inputs["Wv"])
    bq = np.asarray(inputs["bq"], np.float32).astype(np.float16).reshape(H)
    bv = np.asarray(inputs["bv"], np.float32).astype(np.float16).reshape(H)
    cst[:, 3 * CW] = np.tile(bq, 2)
    cst[:, 3 * CW + 1] = np.tile(bv, 2)
    cst = np.ascontiguousarray(cst)
    in_maps = []
    for b in range(B):
        in_maps.append({
            "xq": _stage_xq(q[b]),
            "xk": _stage_x(k[b], NKB, KB),
            "xv": _stage_x(v[b], NKB, KB),
            "cst": cst,
        })
    return in_maps


def kernel(**inputs):
    nc = get_nc()
    in_maps = make_in_maps(inputs)
    res = run_bass_kernel_spmd(nc, in_maps, list(range(B)))
    return np.stack([res.results[b]["out"] for b in range(B)], axis=0)


# revision 24
# speedup vs baseline: 1.0150x; 1.0150x over previous
"""Trainium2 Bass kernel: single attention head (B=8, S=2048, E=1024, H=64).

Sharding: data-parallel over batch -- each of the 8 NeuronCores computes one
batch element's full attention. No collectives; every HBM byte read once.

v12 design (merged constants, max score pairing, qh0-first AV order):
  - v11 post-mortem (82.5us): exp#0 fired at 28us (predicted 18.5) because
    the five tiny weight/bias DMAs each split into 16 per-SDMA-engine
    descriptors whose completion increments trickled out round-robin
    behind the megabyte input packets -- the DVE head-of-line waited on
    bq's sem until 25.9us. Mid-kernel exp gaps (~0.4-2.3us every 2 slabs)
    came from PE in-order queuing: with PE work ~= ACT work (both ~36us),
    AV/proj matmuls ahead of the next score pair push exp out.
  - Constants: ONE f16 blob DMA (wq | wk | wv | bq | bv, [128, 1538]) on
    the scalar ring, done by ~11us with one sem. The duplicated-Wq
    stationary is built on-chip with two cheap DVE copies.
  - Score pairing everywhere timing allows: 14 row-tiled pairs + 4
    singles (PE score cost 10.4 -> 7.7us). Slab order interleaves qh1
    pairs in front of the k3-gated cells so exp never waits on the k3
    arrival; all 16 qh0 cells still finish early so finalize(0/1) and
    the oa-bank recycle overlap the remaining qh1 exps.
  - PSUM (q-half-sequential AV accumulator, as v11): 2x 2-bank f32 score
    slots + 2x 1-bank projection slots + 2-bank [65,1024] AV accumulator
    recycled across q halves = 8 banks. Projections never steal score
    slots.
  - v^T -> vaug via PE transpose + DVE copy (off the DMA rings). Output
    DMAs on the sync ring; scalar engine runs pure ACTIVATE after ~11us.
  - As earlier: transposed scores (keys on partitions), rowsums ride the
    ones column of the AV stationary, bk cancels in softmax, bq/bv fold
    into evacuations, exp scale=1/8 fused into ACTIVATE.
"""

import numpy as np

import concourse.bass as bass  # noqa: F401  (engine namespaces live on nc)
import concourse.mybir as mybir
import concourse.tile as tile
from concourse import bacc
from concourse.bass_utils import run_bass_kernel_spmd
from concourse.masks import make_identity

B, S, E, H = 8, 2048, 1024, 64
EC = E // 128    # contraction chunks (128 partitions each)
KB = 512         # kv block columns
NKB = S // KB    # 4 kv blocks
NT = S // 128    # key tiles
CW = EC * 64     # one weight matrix's columns in the const blob (512)
F16 = mybir.dt.float16
F32 = mybir.dt.float32

_CACHE = {}


def _build_nc():
    nc = bacc.Bacc(None)
    xq = nc.declare_dram_parameter("xq", [128, 2, 2, EC, KB], F16, isOutput=False)
    xk = nc.declare_dram_parameter("xk", [128, NKB, EC, KB], F16, isOutput=False)
    xv = nc.declare_dram_parameter("xv", [128, NKB, EC, KB], F16, isOutput=False)
    cst = nc.declare_dram_parameter("cst", [128, 3 * CW + 2], F16, isOutput=False)
    out = nc.declare_dram_parameter("out", [S, H], F32, isOutput=True)

    Exp = mybir.ActivationFunctionType.Exp

    with tile.TileContext(nc) as tc:
        with tc.tile_pool(name="const", bufs=1) as const, \
             tc.tile_pool(name="xkp", bufs=4) as xkp, \
             tc.tile_pool(name="xvp", bufs=4) as xvp, \
             tc.tile_pool(name="ptp", bufs=16) as ptp, \
             tc.tile_pool(name="vtp", bufs=2) as vtp, \
             tc.tile_pool(name="p5sb", bufs=2) as p5sb, \
             tc.tile_pool(name="psp", bufs=3, space="PSUM") as psp, \
             tc.tile_pool(name="pjp", bufs=1, space="PSUM") as pjp, \
             tc.tile_pool(name="oap", bufs=1, space="PSUM") as oap:

            # ---- constants: ONE blob DMA on the scalar ring ----
            cst_t = const.tile([128, 3 * CW + 2], F16, name="cst_t")
            nc.scalar.dma_start(out=cst_t[:], in_=cst[:])
            cst_r = cst_t[:, 0:3 * CW].rearrange(
                "p (m c w) -> p m c w", m=3, c=EC, w=64)
            bias32 = const.tile([128, 2], F32, name="bias32")
            nc.vector.tensor_copy(bias32[:], cst_t[:, 3 * CW:3 * CW + 2])
            bq_t = bias32[:, 0:1]
            bv_t = bias32[:, 1:2]

            # duplicated-Wq stationary built on-chip (2 cheap DVE copies)
            wqd_t = const.tile([128, EC, 128], F16, name="wqd_t")
            nc.vector.tensor_copy(wqd_t[:, :, 0:64], cst_r[:, 0])
            nc.vector.tensor_copy(wqd_t[:, :, 64:128], cst_r[:, 0])

            def wk_c(c):
                return cst_r[:, 1, c]

            def wv_c(c):
                return cst_r[:, 2, c]

            qt = const.tile([128, S], F16, name="qt")     # q^T in BOTH halves
            kt = const.tile([128, S], F16, name="kt")     # k^T: half (jb%2)
            xqt = const.tile([128, 2, 2, EC, KB], F16, name="xqt")
            vaug = const.tile([128, NT, 80], F16, name="vaug")
            oasb = const.tile([65, S], F16, name="oasb")
            ident = const.tile([128, 128], F16, name="ident")
            osb_all = const.tile([128, NT, H], F32, name="osb_all")

            make_identity(nc, ident[:])
            nc.vector.memset(vaug[:, :, 64], 1.0)

            # AV accumulator: ONE (q-half, seg) quarter at a time
            # ([65, 512] = 1 bank), recycled through 4 rounds with a
            # finalize between -- this frees banks for a 3rd score slot
            oa_tiles = [oap.tile([65, 512], F32, tag="oa", name="oa0")]

            # ---- input DMAs (sync HWDGE FIFO -- executes in this order) ----
            xkts, xvts = [], []

            def fetch(which, jb):
                if which == "k":
                    xt = xkp.tile([128, EC, KB], F16, tag="xk", name=f"xkt{jb}")
                    nc.sync.dma_start(out=xt[:], in_=xk[:, jb])
                    xkts.append(xt)
                else:
                    xt = xvp.tile([128, EC, KB], F16, tag="xv", name=f"xvt{jb}")
                    nc.sync.dma_start(out=xt[:], in_=xv[:, jb])
                    xvts.append(xt)

            nc.sync.dma_start(out=xqt[:, 0, 0], in_=xq[:, 0, 0])
            fetch("k", 0)
            nc.sync.dma_start(out=xqt[:, 0, 1], in_=xq[:, 0, 1])
            fetch("k", 1)
            fetch("k", 2)
            nc.sync.dma_start(out=xqt[:, 1, 0], in_=xq[:, 1, 0])
            nc.sync.dma_start(out=xqt[:, 1, 1], in_=xq[:, 1, 1])
            fetch("v", 0)
            fetch("k", 3)
            fetch("v", 1)
            fetch("v", 2)
            fetch("v", 3)

            # ---- PE warm-keeper: spans the preamble + xq0s0 DMA head so
            # the HAM clock gate is at 2.4 GHz for the first projection.
            wslot = pjp.tile([128, 128], F32, tag="pj", name="warm")
            for _ in range(52):
                nc.tensor.matmul(
                    wslot[:], ident[:], ident[:],
                    start=True, stop=True, skip_group_check=True)

            # ---- AV rounds: one (qh, sg) quarter at a time ----
            pts = [None] * NT           # per-tile exp(S^T) SBUF tiles
            av_done = [[], []]          # per qh: tiles with exp complete
            vdone = set()               # blocks whose vaug tiles exist
            rst = {"r": 0, "drained": set()}

            def emit_av(n):
                # emit up to n AV matmuls of the CURRENT round (each one
                # [65,512], one key tile); eligible = vaug exists + exp'd
                qh, sg = rst["r"] // 2, rst["r"] % 2
                for t in av_done[qh]:
                    if n <= 0:
                        break
                    if t in rst["drained"] or (t // 4) not in vdone:
                        continue
                    cnt = len(rst["drained"])
                    nc.tensor.matmul(
                        oa_tiles[rst["r"]][:],
                        vaug[:, t, 0:65],
                        pts[t][:, qh * 1024 + sg * 512:
                               qh * 1024 + (sg + 1) * 512],
                        start=(cnt == 0), stop=(cnt == NT - 1),
                        skip_group_check=True)
                    rst["drained"].add(t)
                    n -= 1

            def finish_round():
                # drain the round's remaining tiles, then advance
                emit_av(NT)
                assert len(rst["drained"]) == NT, rst
                rst["r"] += 1
                rst["drained"] = set()
                if rst["r"] < 4:
                    oa_tiles.append(oap.tile(
                        [65, 512], F32, tag="oa", name=f"oa{rst['r']}"))

            def qproj_seg(qh, sg):
                # one 512-col q segment: 8 chunk matmuls into a proj slot,
                # then evacuate (+bq) to qt
                ps = pjp.tile([128, KB], F32, tag="pj", name=f"pq{qh}{sg}")
                for c in range(EC):
                    nc.tensor.matmul(
                        ps[:], wqd_t[:, c, :], xqt[:, qh, sg, c, :],
                        start=(c == 0), stop=(c == EC - 1),
                        skip_group_check=True)
                nc.vector.tensor_scalar_add(
                    qt[:, qh * 1024 + sg * 512:qh * 1024 + (sg + 1) * 512],
                    ps[:], bq_t)

            def proj_pass(specs):
                """One col-tiled k/v projection pass in a proj slot.
                specs: list of ("k"|"v", jb, row0), len 1 or 2; row0 in
                {0, 64}, distinct within a pass. k rows MUST equal
                (jb%2)*64 to match kt; v rows are free."""
                ps = pjp.tile([128, KB], F32, tag="pj",
                              name="pkv" + "_".join(f"{w}{j}" for w, j, _ in specs))
                for c in range(EC):
                    for which, jb, r0 in specs:
                        w = wk_c(c) if which == "k" else wv_c(c)
                        x = xkts[jb] if which == "k" else xvts[jb]
                        nc.tensor.matmul(
                            ps[r0:r0 + 64, :], w, x[:, c, :],
                            start=(c == 0), stop=(c == EC - 1),
                            skip_group_check=True)
                vjobs = []
                for which, jb, r0 in specs:
                    if which == "k":
                        assert r0 == (jb % 2) * 64
                        nc.vector.tensor_copy(
                            kt[r0:r0 + 64, jb * KB:(jb + 1) * KB],
                            ps[r0:r0 + 64, :])
                    else:
                        vtb = vtp.tile([128, KB], F16, tag="vt",
                                       name=f"vtb{jb}")
                        nc.vector.tensor_scalar_add(
                            vtb[r0:r0 + 64, :], ps[r0:r0 + 64, :],
                            bv_t[r0:r0 + 64])
                        vjobs.append((jb, r0, vtb))
                for jb, r0, vtb in vjobs:
                    # v^T [64, 512] -> vaug 4x[128, 64] via PE transpose
                    # (stays off the DMA rings); trs2 takes a proj slot
                    trs2 = pjp.tile([128, 4, H], F16, tag="pj",
                                    name=f"vtr{jb}")
                    for j in range(4):
                        nc.tensor.transpose(
                            trs2[:, j, 0:H],
                            vtb[r0:r0 + 64, j * 128:(j + 1) * 128],
                            ident[r0:r0 + 64, r0:r0 + 64])
                    for j in range(4):
                        nc.vector.tensor_copy(
                            vaug[:, 4 * jb + j, 0:H], trs2[:, j, 0:H])
                    vdone.add(jb)

            def score_seg(t, qh, sl, sg, exp=True):
                # one N=512 score matmul (+ optional FD-512 exp) for seg sg
                g = ((t // 4) % 2) * 64
                nc.tensor.matmul(
                    sl[:, sg * 512:(sg + 1) * 512],
                    kt[g:g + 64, t * 128:(t + 1) * 128],
                    qt[g:g + 64, qh * 1024 + sg * 512:
                       qh * 1024 + (sg + 1) * 512],
                    start=True, stop=True, skip_group_check=True)
                if exp:
                    nc.scalar.activation(
                        pts[t][:, qh * 1024 + sg * 512:
                               qh * 1024 + (sg + 1) * 512],
                        sl[:, sg * 512:(sg + 1) * 512], Exp, scale=0.125)
                    if sg == 1:
                        av_done[qh].append(t)

            def slab_tile(t, qh):
                if pts[t] is None:
                    pts[t] = ptp.tile([128, S], F16, tag="pt", name=f"pt{t}")
                return psp.tile([128, 1024], F32, tag="ps", name=f"s{t}_{qh}")

            def score_slabs(cells, av=0):
                """Scores + exp for 1 or 2 (tile, qh) cells: 2x N=512
                matmuls into a 2-bank f32 slab + one FD-1024 exp each.
                A len-2 list must have opposite kt-half parity; its four
                matmuls interleave seg-by-seg so the two cells run
                concurrently on the PE via row tiling."""
                work = [(t, qh, slab_tile(t, qh)) for t, qh in cells]
                for sg in range(2):
                    for t, qh, sl in work:
                        score_seg(t, qh, sl, sg, exp=False)
                for t, qh, sl in work:
                    nc.scalar.activation(
                        pts[t][:, qh * 1024:(qh + 1) * 1024], sl[:],
                        Exp, scale=0.125)
                    av_done[qh].append(t)
                if av:
                    emit_av(av)

            # ---- finalize: transpose, normalize, store (out on sync ring)
            out_r = out[:].rearrange("(t p) h -> p t h", p=128)

            def finalize_chunk(cq):
                nc.vector.tensor_copy(
                    oasb[:, cq * 512:(cq + 1) * 512], oa_tiles[cq][:])
                trs = pjp.tile([128, 4, 66], F16, tag="pj", name=f"trs{cq}")
                for jj in range(4):
                    j = cq * 4 + jj
                    nc.tensor.transpose(
                        trs[:, jj, 0:65], oasb[:, j * 128:(j + 1) * 128],
                        ident[0:65, 0:65])
                rc = p5sb.tile([128, 4], F32, tag="rc", name=f"rc{cq}")
                nc.vector.reciprocal(rc[:], trs[:, :, 64])
                for jj in range(4):
                    j = cq * 4 + jj
                    nc.vector.tensor_scalar(
                        osb_all[:, j, :], trs[:, jj, 0:64], rc[:, jj:jj + 1],
                        None, op0=mybir.AluOpType.mult)
                nc.sync.dma_start(
                    out=out_r[:, cq * 4:(cq + 1) * 4, :],
                    in_=osb_all[:, cq * 4:(cq + 1) * 4, :])

            # ---- schedule (program order ~= per-engine issue order) ----
            # exp slab cadence ~1.15us; stream arrival gates annotated.
            qproj_seg(0, 0)                    # xq0s0
            proj_pass([("k", 0, 0)])           # k0
            sl00 = slab_tile(0, 0)             # cell (0,0) split by seg:
            score_seg(0, 0, sl00, 0)           # exp starts before xq0s1
            qproj_seg(0, 1)                    # xq0s1
            score_seg(0, 0, sl00, 1)
            score_slabs([(1, 0)])
            proj_pass([("k", 1, 64)])          # k1
            score_slabs([(4, 0), (2, 0)])      # row-tiled pairs (h1 x h0)
            proj_pass([("k", 2, 0)])           # k2
            score_slabs([(5, 0), (3, 0)])
            score_slabs([(6, 0), (8, 0)])
            score_slabs([(7, 0), (9, 0)])
            qproj_seg(1, 0)                    # xq1s0
            qproj_seg(1, 1)                    # xq1s1
            score_slabs([(4, 1), (0, 1)])
            proj_pass([("v", 0, 0)])           # v0
            score_slabs([(5, 1), (1, 1)], av=2)
            proj_pass([("k", 3, 64)])          # k3
            score_slabs([(6, 1), (8, 1)], av=2)
            score_slabs([(7, 1), (9, 1)], av=2)
            proj_pass([("v", 1, 0)])           # v1
            score_slabs([(12, 0), (10, 0)], av=3)
            score_slabs([(13, 0), (11, 0)], av=3)
            proj_pass([("v", 2, 0)])           # v2
            score_slabs([(14, 0), (2, 1)], av=3)
            proj_pass([("v", 3, 64)])          # v3
            score_slabs([(15, 0), (3, 1)], av=3)
            # round 0 = (qh0, sg0) complete once (15,0)'s exp lands;
            # finalize chunk 0 and recycle the bank for round 1, all
            # overlapped with the remaining qh1 exps
            finish_round()
            finalize_chunk(0)
            score_slabs([(12, 1), (10, 1)], av=6)
            score_slabs([(13, 1), (11, 1)], av=6)
            finish_round()                     # (qh0, sg1)
            finalize_chunk(1)
            score_slabs([(14, 1)], av=6)
            score_slabs([(15, 1)], av=6)
            finish_round()                     # (qh1, sg0)
            finalize_chunk(2)
            finish_round()                     # (qh1, sg1)
            finalize_chunk(3)

    nc.finalize()
    return nc


def get_nc():
    if "nc" not in _CACHE:
        _CACHE["nc"] = _build_nc()
    return _CACHE["nc"]


def _stage_x(x, nblk, cb):
    # [S, E] f32 -> [128, nblk, EC, cb] f16 with [p, b, c, s] = x[b*cb+s, c*128+p]
    xt = np.ascontiguousarray(x.T.astype(np.float16))          # [E, S]
    xt = xt.reshape(EC, 128, nblk, cb).transpose(1, 2, 0, 3)   # [p, b, c, s]
    return np.ascontiguousarray(xt)


def _stage_xq(x):
    # [S, E] f32 -> [128, 2, 2, EC, 512] f16, seg-major:
    # [p, h, s, c, s2] = x[h*1024 + s*512 + s2, c*128 + p]
    xt = np.ascontiguousarray(x.T.astype(np.float16))          # [E, S]
    xt = xt.reshape(EC, 128, 2, 2, KB).transpose(1, 2, 3, 0, 4)
    return np.ascontiguousarray(xt)


def _stage_w(w):
    # [E, 64] -> [128, EC*64] f16 with [p, c*64+j] = w[c*128+p, j]
    wh = np.asarray(w, np.float32).astype(np.float16)
    return wh.reshape(EC, 128, 64).transpose(1, 0, 2).reshape(128, CW)


def make_in_maps(inputs):
    q = np.asarray(inputs["query"], np.float32)
    k = np.asarray(inputs["key_"], np.float32)
    v = np.asarray(inputs["value"], np.float32)
    cst = np.zeros((128, 3 * CW + 2), np.float16)
    cst[:, 0:CW] = _stage_w(inputs["Wq"])
    cst[:, CW:2 * CW] = _stage_w(inputs["Wk"])
    cst[:, 2 * CW:3 * CW] = _stage_w(inputs["Wv"])
    bq = np.asarray(inputs["bq"], np.float32).astype(np.float16).reshape(H)
    bv = np.asarray(inputs["bv"], np.float32).astype(np.float16).reshape(H)
    cst[:, 3 * CW] = np.tile(bq, 2)
    cst[:, 3 * CW + 1] = np.tile(bv, 2)
    cst = np.ascontiguousarray(cst)
    in_maps = []
    for b in range(B):
        in_maps.append({
            "xq": _stage_xq(q[b]),
            "xk": _stage_x(k[b], NKB, KB),
            "xv": _stage_x(v[b], NKB, KB),
            "cst": cst,
        })
    return in_maps


def kernel(**inputs):
    nc = get_nc()
    in_maps = make_in_maps(inputs)
    res = run_bass_kernel_spmd(nc, in_maps, list(range(B)))
    return np.stack([res.results[b]["out"] for b in range(B)], axis=0)


# revision 29
# speedup vs baseline: 1.0487x; 1.0332x over previous
"""Trainium2 Bass kernel: single attention head (B=8, S=2048, E=1024, H=64).

Sharding: data-parallel over batch -- each of the 8 NeuronCores computes one
batch element's full attention. No collectives; every HBM byte read once.

v12 design (merged constants, max score pairing, qh0-first AV order):
  - v11 post-mortem (82.5us): exp#0 fired at 28us (predicted 18.5) because
    the five tiny weight/bias DMAs each split into 16 per-SDMA-engine
    descriptors whose completion increments trickled out round-robin
    behind the megabyte input packets -- the DVE head-of-line waited on
    bq's sem until 25.9us. Mid-kernel exp gaps (~0.4-2.3us every 2 slabs)
    came from PE in-order queuing: with PE work ~= ACT work (both ~36us),
    AV/proj matmuls ahead of the next score pair push exp out.
  - Constants: ONE f16 blob DMA (wq | wk | wv | bq | bv, [128, 1538]) on
    the scalar ring, done by ~11us with one sem. The duplicated-Wq
    stationary is built on-chip with two cheap DVE copies.
  - Score pairing everywhere timing allows: 14 row-tiled pairs + 4
    singles (PE score cost 10.4 -> 7.7us). Slab order interleaves qh1
    pairs in front of the k3-gated cells so exp never waits on the k3
    arrival; all 16 qh0 cells still finish early so finalize(0/1) and
    the oa-bank recycle overlap the remaining qh1 exps.
  - PSUM (q-half-sequential AV accumulator, as v11): 2x 2-bank f32 score
    slots + 2x 1-bank projection slots + 2-bank [65,1024] AV accumulator
    recycled across q halves = 8 banks. Projections never steal score
    slots.
  - v^T -> vaug via PE transpose + DVE copy (off the DMA rings). Output
    DMAs on the sync ring; scalar engine runs pure ACTIVATE after ~11us.
  - As earlier: transposed scores (keys on partitions), rowsums ride the
    ones column of the AV stationary, bk cancels in softmax, bq/bv fold
    into evacuations, exp scale=1/8 fused into ACTIVATE.
"""

import numpy as np

import concourse.bass as bass  # noqa: F401  (engine namespaces live on nc)
import concourse.mybir as mybir
import concourse.tile as tile
from concourse import bacc
from concourse.bass_utils import run_bass_kernel_spmd
from concourse.masks import make_identity

B, S, E, H = 8, 2048, 1024, 64
EC = E // 128    # contraction chunks (128 partitions each)
KB = 512         # kv block columns
NKB = S // KB    # 4 kv blocks
NT = S // 128    # key tiles
CW = EC * 64     # one weight matrix's columns in the const blob (512)
F16 = mybir.dt.float16
F32 = mybir.dt.float32

_CACHE = {}


def _build_nc():
    nc = bacc.Bacc(None)
    xq = nc.declare_dram_parameter("xq", [128, 2, 2, EC, KB], F16, isOutput=False)
    xk = nc.declare_dram_parameter("xk", [128, NKB, EC, KB], F16, isOutput=False)
    xv = nc.declare_dram_parameter("xv", [128, NKB, EC, KB], F16, isOutput=False)
    cst = nc.declare_dram_parameter("cst", [128, 3 * CW + 2], F16, isOutput=False)
    out = nc.declare_dram_parameter("out", [S, H], F32, isOutput=True)

    Exp = mybir.ActivationFunctionType.Exp

    with tile.TileContext(nc) as tc:
        with tc.tile_pool(name="const", bufs=1) as const, \
             tc.tile_pool(name="xkp", bufs=4) as xkp, \
             tc.tile_pool(name="xvp", bufs=4) as xvp, \
             tc.tile_pool(name="ptp", bufs=16) as ptp, \
             tc.tile_pool(name="vtp", bufs=2) as vtp, \
             tc.tile_pool(name="p5sb", bufs=2) as p5sb, \
             tc.tile_pool(name="psp", bufs=3, space="PSUM") as psp, \
             tc.tile_pool(name="pjp", bufs=1, space="PSUM") as pjp, \
             tc.tile_pool(name="oap", bufs=1, space="PSUM") as oap:

            # ---- constants: ONE blob DMA on the scalar ring ----
            cst_t = const.tile([128, 3 * CW + 2], F16, name="cst_t")
            nc.scalar.dma_start(out=cst_t[:], in_=cst[:])
            cst_r = cst_t[:, 0:3 * CW].rearrange(
                "p (m c w) -> p m c w", m=3, c=EC, w=64)
            bias32 = const.tile([128, 2], F32, name="bias32")
            nc.vector.tensor_copy(bias32[:], cst_t[:, 3 * CW:3 * CW + 2])
            bq_t = bias32[:, 0:1]
            bv_t = bias32[:, 1:2]

            # duplicated-Wq stationary built on-chip (2 cheap DVE copies)
            wqd_t = const.tile([128, EC, 128], F16, name="wqd_t")
            nc.vector.tensor_copy(wqd_t[:, :, 0:64], cst_r[:, 0])
            nc.vector.tensor_copy(wqd_t[:, :, 64:128], cst_r[:, 0])

            def wk_c(c):
                return cst_r[:, 1, c]

            def wv_c(c):
                return cst_r[:, 2, c]

            qt = const.tile([128, S], F16, name="qt")     # q^T in BOTH halves
            kt = const.tile([128, S], F16, name="kt")     # k^T: half (jb%2)
            xqt = const.tile([128, 2, 2, EC, KB], F16, name="xqt")
            vaug = const.tile([128, NT, 80], F16, name="vaug")
            oasb = const.tile([65, S], F16, name="oasb")
            ident = const.tile([128, 128], F16, name="ident")
            osb_all = const.tile([128, NT, H], F32, name="osb_all")

            make_identity(nc, ident[:])
            nc.vector.memset(vaug[:, :, 64], 1.0)

            # AV accumulator: ONE (q-half, seg) quarter at a time
            # ([65, 512] = 1 bank), recycled through 4 rounds with a
            # finalize between -- this frees banks for a 3rd score slot
            oa_tiles = [oap.tile([65, 512], F32, tag="oa", name="oa0")]

            # ---- input DMAs (sync HWDGE FIFO -- executes in this order) ----
            xkts, xvts = [], []

            def fetch(which, jb):
                if which == "k":
                    xt = xkp.tile([128, EC, KB], F16, tag="xk", name=f"xkt{jb}")
                    nc.sync.dma_start(out=xt[:], in_=xk[:, jb])
                    xkts.append(xt)
                else:
                    xt = xvp.tile([128, EC, KB], F16, tag="xv", name=f"xvt{jb}")
                    nc.sync.dma_start(out=xt[:], in_=xv[:, jb])
                    xvts.append(xt)

            nc.sync.dma_start(out=xqt[:, 0, 0], in_=xq[:, 0, 0])
            fetch("k", 0)
            nc.sync.dma_start(out=xqt[:, 0, 1], in_=xq[:, 0, 1])
            fetch("k", 1)
            fetch("k", 2)
            nc.sync.dma_start(out=xqt[:, 1, 0], in_=xq[:, 1, 0])
            nc.sync.dma_start(out=xqt[:, 1, 1], in_=xq[:, 1, 1])
            fetch("v", 0)
            fetch("k", 3)
            fetch("v", 1)
            fetch("v", 2)
            fetch("v", 3)

            # ---- PE warm-keeper: spans the preamble + xq0s0 DMA head so
            # the HAM clock gate is at 2.4 GHz for the first projection.
            wslot = pjp.tile([128, 128], F32, tag="pj", name="warm")
            for _ in range(64):
                nc.tensor.matmul(
                    wslot[:], ident[:], ident[:],
                    start=True, stop=True, skip_group_check=True)

            # ---- AV rounds: one (qh, sg) quarter at a time ----
            pts = [None] * NT           # per-tile exp(S^T) SBUF tiles
            av_done = [[], []]          # per qh: tiles with exp complete
            vdone = set()               # blocks whose vaug tiles exist
            rst = {"r": 0, "drained": set()}

            def emit_av(n):
                # emit up to n AV matmuls of the CURRENT round (each one
                # [65,512], one key tile); eligible = vaug exists + exp'd
                qh, sg = rst["r"] // 2, rst["r"] % 2
                for t in av_done[qh]:
                    if n <= 0:
                        break
                    if t in rst["drained"] or (t // 4) not in vdone:
                        continue
                    cnt = len(rst["drained"])
                    nc.tensor.matmul(
                        oa_tiles[rst["r"]][:],
                        vaug[:, t, 0:65],
                        pts[t][:, qh * 1024 + sg * 512:
                               qh * 1024 + (sg + 1) * 512],
                        start=(cnt == 0), stop=(cnt == NT - 1),
                        skip_group_check=True)
                    rst["drained"].add(t)
                    n -= 1

            def finish_round():
                # drain the round's remaining tiles, then advance
                emit_av(NT)
                assert len(rst["drained"]) == NT, rst
                rst["r"] += 1
                rst["drained"] = set()
                if rst["r"] < 4:
                    oa_tiles.append(oap.tile(
                        [65, 512], F32, tag="oa", name=f"oa{rst['r']}"))

            Copy = mybir.ActivationFunctionType.Copy

            def qproj_seg(qh, sg, evac_engine="v"):
                # one 512-col q segment: 8 chunk matmuls into a proj slot,
                # then evacuate (+bq) to qt. Early passes evacuate on the
                # (still idle) ScalarE so the single proj slot frees fast.
                ps = pjp.tile([128, KB], F32, tag="pj", name=f"pq{qh}{sg}")
                for c in range(EC):
                    nc.tensor.matmul(
                        ps[:], wqd_t[:, c, :], xqt[:, qh, sg, c, :],
                        start=(c == 0), stop=(c == EC - 1),
                        skip_group_check=True)
                dst = qt[:, qh * 1024 + sg * 512:qh * 1024 + (sg + 1) * 512]
                if evac_engine == "s":
                    nc.scalar.activation(dst, ps[:], Copy, bias=bq_t)
                else:
                    nc.vector.tensor_scalar_add(dst, ps[:], bq_t)

            def proj_pass(specs):
                """One col-tiled k/v projection pass in a proj slot.
                specs: list of ("k"|"v", jb, row0), len 1 or 2; row0 in
                {0, 64}, distinct within a pass. k rows MUST equal
                (jb%2)*64 to match kt; v rows are free."""
                ps = pjp.tile([128, KB], F32, tag="pj",
                              name="pkv" + "_".join(f"{w}{j}" for w, j, _ in specs))
                for c in range(EC):
                    for which, jb, r0 in specs:
                        w = wk_c(c) if which == "k" else wv_c(c)
                        x = xkts[jb] if which == "k" else xvts[jb]
                        nc.tensor.matmul(
                            ps[r0:r0 + 64, :], w, x[:, c, :],
                            start=(c == 0), stop=(c == EC - 1),
                            skip_group_check=True)
                vjobs = []
                for which, jb, r0 in specs:
                    if which == "k":
                        assert r0 == (jb % 2) * 64
                        dst = kt[r0:r0 + 64, jb * KB:(jb + 1) * KB]
                        if jb == 0:
                            # pre-exp: ScalarE is idle, evacuate there
                            nc.scalar.activation(dst, ps[r0:r0 + 64, :], Copy)
                        else:
                            nc.vector.tensor_copy(dst, ps[r0:r0 + 64, :])
                    else:
                        vtb = vtp.tile([128, KB], F16, tag="vt",
                                       name=f"vtb{jb}")
                        nc.vector.tensor_scalar_add(
                            vtb[r0:r0 + 64, :], ps[r0:r0 + 64, :],
                            bv_t[r0:r0 + 64])
                        vjobs.append((jb, r0, vtb))
                for jb, r0, vtb in vjobs:
                    # v^T [64, 512] -> vaug 4x[128, 64] via PE transpose
                    # (stays off the DMA rings); trs2 takes a proj slot
                    trs2 = pjp.tile([128, 4, H], F16, tag="pj",
                                    name=f"vtr{jb}")
                    for j in range(4):
                        nc.tensor.transpose(
                            trs2[:, j, 0:H],
                            vtb[r0:r0 + 64, j * 128:(j + 1) * 128],
                            ident[r0:r0 + 64, r0:r0 + 64])
                    for j in range(4):
                        nc.vector.tensor_copy(
                            vaug[:, 4 * jb + j, 0:H], trs2[:, j, 0:H])
                    vdone.add(jb)

            def score_seg(t, qh, sl, sg, exp=True):
                # one N=512 score matmul (+ optional FD-512 exp) for seg sg
                g = ((t // 4) % 2) * 64
                nc.tensor.matmul(
                    sl[:, sg * 512:(sg + 1) * 512],
                    kt[g:g + 64, t * 128:(t + 1) * 128],
                    qt[g:g + 64, qh * 1024 + sg * 512:
                       qh * 1024 + (sg + 1) * 512],
                    start=True, stop=True, skip_group_check=True)
                if exp:
                    nc.scalar.activation(
                        pts[t][:, qh * 1024 + sg * 512:
                               qh * 1024 + (sg + 1) * 512],
                        sl[:, sg * 512:(sg + 1) * 512], Exp, scale=0.125)
                    if sg == 1:
                        av_done[qh].append(t)

            def slab_tile(t, qh):
                if pts[t] is None:
                    pts[t] = ptp.tile([128, S], F16, tag="pt", name=f"pt{t}")
                return psp.tile([128, 1024], F32, tag="ps", name=f"s{t}_{qh}")

            def score_slabs(cells, av=0):
                """Scores + exp for 1 or 2 (tile, qh) cells: 2x N=512
                matmuls into a 2-bank f32 slab + one FD-1024 exp each.
                A len-2 list must have opposite kt-half parity; its four
                matmuls interleave seg-by-seg so the two cells run
                concurrently on the PE via row tiling."""
                work = [(t, qh, slab_tile(t, qh)) for t, qh in cells]
                for sg in range(2):
                    for t, qh, sl in work:
                        score_seg(t, qh, sl, sg, exp=False)
                for t, qh, sl in work:
                    nc.scalar.activation(
                        pts[t][:, qh * 1024:(qh + 1) * 1024], sl[:],
                        Exp, scale=0.125)
                    av_done[qh].append(t)
                if av:
                    emit_av(av)

            # ---- finalize: transpose, normalize, store (out on sync ring)
            out_r = out[:].rearrange("(t p) h -> p t h", p=128)

            def finalize_chunk(cq):
                nc.vector.tensor_copy(
                    oasb[:, cq * 512:(cq + 1) * 512], oa_tiles[cq][:])
                trs = pjp.tile([128, 4, 66], F16, tag="pj", name=f"trs{cq}")
                for jj in range(4):
                    j = cq * 4 + jj
                    nc.tensor.transpose(
                        trs[:, jj, 0:65], oasb[:, j * 128:(j + 1) * 128],
                        ident[0:65, 0:65])
                rc = p5sb.tile([128, 4], F32, tag="rc", name=f"rc{cq}")
                nc.vector.reciprocal(rc[:], trs[:, :, 64])
                for jj in range(4):
                    j = cq * 4 + jj
                    nc.vector.tensor_scalar(
                        osb_all[:, j, :], trs[:, jj, 0:64], rc[:, jj:jj + 1],
                        None, op0=mybir.AluOpType.mult)
                nc.sync.dma_start(
                    out=out_r[:, cq * 4:(cq + 1) * 4, :],
                    in_=osb_all[:, cq * 4:(cq + 1) * 4, :])

            # ---- schedule (program order ~= per-engine issue order) ----
            # exp slab cadence ~1.15us; stream arrival gates annotated.
            qproj_seg(0, 0)                    # xq0s0
            proj_pass([("k", 0, 0)])           # k0
            sl00 = slab_tile(0, 0)             # cell (0,0) split by seg:
            score_seg(0, 0, sl00, 0)           # exp starts before xq0s1
            qproj_seg(0, 1)                    # xq0s1
            score_seg(0, 0, sl00, 1)
            score_slabs([(1, 0)])
            proj_pass([("k", 1, 64)])          # k1
            score_slabs([(4, 0), (2, 0)])      # row-tiled pairs (h1 x h0)
            proj_pass([("k", 2, 0)])           # k2
            score_slabs([(5, 0), (3, 0)])
            score_slabs([(6, 0), (8, 0)])
            score_slabs([(7, 0), (9, 0)])
            qproj_seg(1, 0)                    # xq1s0
            qproj_seg(1, 1)                    # xq1s1
            score_slabs([(4, 1), (0, 1)])
            proj_pass([("v", 0, 0)])           # v0
            score_slabs([(5, 1), (1, 1)], av=2)
            proj_pass([("k", 3, 64)])          # k3
            score_slabs([(6, 1), (8, 1)], av=2)
            score_slabs([(7, 1), (9, 1)], av=2)
            proj_pass([("v", 1, 0)])           # v1
            score_slabs([(12, 0), (10, 0)], av=3)
            score_slabs([(13, 0), (11, 0)], av=3)
            proj_pass([("v", 2, 0)])           # v2
            score_slabs([(14, 0), (2, 1)], av=3)
            proj_pass([("v", 3, 64)])          # v3
            score_slabs([(15, 0), (3, 1)], av=3)
            # round 0 = (qh0, sg0) complete once (15,0)'s exp lands;
            # finalize chunk 0 and recycle the bank for round 1, all
            # overlapped with the remaining qh1 exps
            finish_round()
            finalize_chunk(0)
            score_slabs([(12, 1), (10, 1)], av=6)
            score_slabs([(13, 1), (11, 1)], av=6)
            finish_round()                     # (qh0, sg1)
            finalize_chunk(1)
            score_slabs([(14, 1)], av=6)
            score_slabs([(15, 1)], av=6)
            finish_round()                     # (qh1, sg0)
            finalize_chunk(2)
            finish_round()                     # (qh1, sg1)
            finalize_chunk(3)

    nc.finalize()
    return nc


def get_nc():
    if "nc" not in _CACHE:
        _CACHE["nc"] = _build_nc()
    return _CACHE["nc"]


def _stage_x(x, nblk, cb):
    # [S, E] f32 -> [128, nblk, EC, cb] f16 with [p, b, c, s] = x[b*cb+s, c*128+p]
    xt = np.ascontiguousarray(x.T.astype(np.float16))          # [E, S]
    xt = xt.reshape(EC, 128, nblk, cb).transpose(1, 2, 0, 3)   # [p, b, c, s]
    return np.ascontiguousarray(xt)


def _stage_xq(x):
    # [S, E] f32 -> [128, 2, 2, EC, 512] f16, seg-major:
    # [p, h, s, c, s2] = x[h*1024 + s*512 + s2, c*128 + p]
    xt = np.ascontiguousarray(x.T.astype(np.float16))          # [E, S]
    xt = xt.reshape(EC, 128, 2, 2, KB).transpose(1, 2, 3, 0, 4)
    return np.ascontiguousarray(xt)


def _stage_w(w):
    # [E, 64] -> [128, EC*64] f16 with [p, c*64+j] = w[c*128+p, j]
    wh = np.asarray(w, np.float32).astype(np.float16)
    return wh.reshape(EC, 128, 64).transpose(1, 0, 2).reshape(128, CW)


def make_in_maps(inputs):
    q = np.asarray(inputs["query"], np.float32)
    k = np.asarray(inputs["key_"], np.float32)
    v = np.asarray(inputs["value"], np.float32)
    cst = np.zeros((128, 3 * CW + 2), np.float16)
    cst[:, 0:CW] = _stage_w(inputs["Wq"])
    cst[:, CW:2 * CW] = _stage_w(inputs["Wk"])
    cst[:, 2 * CW:3 * CW] = _stage_w(inputs["Wv"])
    bq = np.asarray(inputs["bq"], np.float32).astype(np.float16).reshape(H)
    bv = np.asarray(inputs["bv"], np.float32).astype(np.float16).reshape(H)
    cst[:, 3 * CW] = np.tile(bq, 2)
    cst[:, 3 * CW + 1] = np.tile(bv, 2)
    cst = np.ascontiguousarray(cst)
    in_maps = []
    for b in range(B):
        in_maps.append({
            "xq": _stage_xq(q[b]),
            "xk": _stage_x(k[b], NKB, KB),
            "xv": _stage_x(v[b], NKB, KB),
            "cst": cst,
        })
    return in_maps


def kernel(**inputs):
    nc = get_nc()
    in_maps = make_in_maps(inputs)
    res = run_bass_kernel_spmd(nc, in_maps, list(range(B)))
    return np.stack([res.results[b]["out"] for b in range(B)], axis=0)
